# revision 40
# baseline (speedup 1.0000x reference)
"""Trainium2 Bass kernel for the DfOp deep-filtering module.

out[b, t, f<96]  = sum_{k=0..4} coefs[b, k, t, f] (*) spec[b, t-4+k, f]   (complex mult)
out[b, t, f>=96] = spec[b, t, f]                                          (passthrough)

Sharding: data-parallel over batch B=8 -> one batch element per NeuronCore.

The hi band (385 of 481 bins) is a pure passthrough, merged on the host
during gather; it never touches the device.  The device computes only the
96-bin lo band from HOST-PREPACKED planar fp16 (planes de-interleaved, im
coef plane pre-negated, causal halo prepacked per partition):

  spec  [2(piece), 2(plane), 128, 20*96]  piece 0 = window rows 0:20,
        piece 1 = rows 16:36 (4 rows duplicated so each 16-row product
        half-block reads exactly one piece -> whole-tile dependencies)
  coefs [5(tap), 2(half), 2(plane), 128, 16*96]  plane 1 = -c_im; the
        (re | -im) pair of one (tap, half) is one load / one semaphore,
        with 3072B-contiguous DRAM rows per partition (SDMA packet
        cadence is ~fixed per row, so wide rows = full bandwidth)
  coef00 [4(rowchunk), 128, 2(plane)*4*96]  tap-0 half-0 duplicated in
        row-chunk-major quarters so the ramp chunk loads also get wide
        contiguous rows
  consts [128, 256] = [I | -I] fp16 identity weights, single DMA
  out   [4096, 192] fp16   row t = [re(96) | im(96)], split on host

Schedule (from trace analysis):
  - exec = ~7us fixed preamble + ramp + dense DVE products + tail.
  - Ramp: single Sync-ring load stream in exact DVE consumption order (a
    second concurrent ring delays the FIRST loads via packet round-robin);
    tap-0 runs 4-row products ordered so the first 4 need only loads 1-2.
  - Mid: all products on DVE (GpSimd/Pool measured 3.7us/op AND slows
    concurrent DVE tensor_tensor ~28% via the shared SBUF port pair -- do
    not use; ACT cannot read two tensor streams).  h1 tap pairs 1+2 and
    3+4 are merged into double-width products via a tap-strided spec AP
    (rows overlap, tap stride = one 96-col row).  PE accumulates via
    resident fp16 +/-identity into fp32 PSUM (512-col-max moving operand).
    Note: the mid-kernel is input-DMA-paced (~300 GB/s effective/core with
    all 8 cores streaming), so further DVE reductions buy slack, not time.
  - Tail: the last tap's products run chunk 3 first so its PSUM stop +
    drain + store overlap the chunk-2 products; chunk 2 drains+stores in
    4-row halves, ACT taking half 0 and DVE half 1 in parallel.
"""

import sys

import numpy as np

try:
    import concourse.bacc  # noqa: F401  (resolves via the environment's path)
except ImportError:  # pragma: no cover - fallback for bare environments
    for _p in ("/opt/trn_rl_repo", "/root/.axon_site/_ro/trn_rl_repo"):
        if _p not in sys.path:
            sys.path.append(_p)

import bass_rust as _bass_rust
import concourse.bacc as bacc
import concourse.mybir as mybir
from concourse.tile import TileContext
from concourse.bass_utils import run_bass_kernel_spmd

B = 8          # batch / cores
T = 4096       # time steps
F = 481        # total freq bins
NF = 96        # deep-filtered freq bins
FS = 5         # frame size (causal taps)
HL = FS - 1    # halo slots (4)
P = 128        # partitions
TB = T // P    # timesteps per partition block   (32)
HALF = TB // 2                 # 16 rows per product half-block
SROWS = HALF + HL              # 20 rows per spec piece
WIN = TB + HL                  # 36 window rows (pieces at 0 and HALF)
CHUNK = 8                      # PSUM chunk rows
CW = CHUNK * NF                # 768 psum cols per re/im region
SC = SROWS * NF                # 1920 spec cols per plane
KC = HALF * NF                 # 1536 coef cols per plane
JC = CHUNK * NF                # 768 coef cols per (plane, rowchunk)
MERGED = ((1, 1), (2, 1))      # (tap, half) pair computed as one product
MERGED34 = ((3, 1), (4, 1))    # second merged pair; chunk-split for the tail

_nc_cache = None


def _body(nc, tc, spec_d, coef_d, coef00_d, const_d, out_d):
    f16 = mybir.dt.float16
    f32 = mybir.dt.float32

    outv = out_d.rearrange("(q i) u -> q i u", i=TB)            # [128, 32, 192]

    with (
        tc.tile_pool(name="const", bufs=1) as cpool,
        tc.tile_pool(name="spec", bufs=1) as spool,
        tc.tile_pool(name="coef", bufs=5) as kpool,
        tc.tile_pool(name="coefw", bufs=1) as kwpool,
        tc.tile_pool(name="prod", bufs=20) as ppool,
        tc.tile_pool(name="prodw", bufs=8) as pwpool,
        tc.tile_pool(name="out", bufs=4) as opool,
        tc.tile_pool(name="psum", bufs=2, space="PSUM") as pspool,
    ):
        # [I | -I] weights first on the ACT ring
        cc = cpool.tile([P, 2 * P], f16)
        nc.scalar.dma_start(out=cc[:], in_=const_d)
        ident = cc[:, 0:P]
        identn = cc[:, P:2 * P]

        # piece-0 spec planes as separate tiles; piece-1 planes paired
        s0 = [spool.tile([P, SC], f16, tag=f"spec0{c}", name=f"spec0{c}")
              for c in range(2)]
        s1 = spool.tile([P, 2 * SC], f16, tag="spec1", name="spec1")

        def spec_cols(h, c):
            return s0[c][:] if h == 0 else s1[:, c * SC:(c + 1) * SC]

        # coef tiles: tap0-h0 rowchunk-major [j c i f]; merged h1 taps 1+2
        # tap-major [k c i f]; everything else plane-major [c i f]
        ct00 = kwpool.tile([P, 2 * KC], f16, tag="coef00", name="coef00")
        ctm = kwpool.tile([P, 4 * KC], f16, tag="coefm", name="coefm")
        ctm34 = kwpool.tile([P, 4 * KC], f16, tag="coefm34", name="coefm34")
        ctiles = {}
        for k in range(FS):
            for h in range(2):
                if (k, h) == (0, 0) or (k, h) in MERGED + MERGED34:
                    continue
                ctiles[(k, h)] = kpool.tile([P, 2 * KC], f16, tag="coef",
                                            name=f"coef{k}{h}")

        def load_coef(k, h):
            ct = ctiles[(k, h)]
            nc.sync.dma_start(out=ct[:].rearrange("p (c f) -> p c f", c=2),
                              in_=coef_d[k, h].rearrange("c p f -> p c f"))

        # loads (Sync ring, FIFO) in exact DVE consumption order.  The ramp
        # is data-arrival-bound (tap 1 cannot start before ~2.4MB lands), so
        # the first products are gated just-in-time rather than as-early-as-
        # possible -- smaller first loads only convert the wait into gaps.
        nc.sync.dma_start(out=s0[0][:], in_=spec_d[0, 0])        # s_re plane
        nc.sync.dma_start(                                       # c0 rows 0:8
            out=ct00[:, 0:2 * JC].rearrange("p (j x) -> p j x", j=2),
            in_=coef00_d[0:2].rearrange("j p x -> p j x"))
        nc.sync.dma_start(out=s0[1][:], in_=spec_d[0, 1])        # s_im plane
        nc.sync.dma_start(                                       # c0 rows 8:16
            out=ct00[:, 2 * JC:4 * JC].rearrange("p (j x) -> p j x", j=2),
            in_=coef00_d[2:4].rearrange("j p x -> p j x"))
        for k in range(1, FS):
            load_coef(k, 0)
        nc.sync.dma_start(out=s1[:].rearrange("p (c f) -> p c f", c=2),
                          in_=spec_d[1].rearrange("c p f -> p c f"))
        load_coef(0, 1)
        for ki, k in enumerate((1, 2)):                          # taps 1+2 h1
            nc.sync.dma_start(
                out=ctm[:, ki * 2 * KC:(ki + 1) * 2 * KC].rearrange(
                    "p (c f) -> p c f", c=2),
                in_=coef_d[k, 1].rearrange("c p f -> p c f"))
        for ki, k in enumerate((3, 4)):                          # taps 3+4 h1
            nc.sync.dma_start(
                out=ctm34[:, ki * 2 * KC:(ki + 1) * 2 * KC].rearrange(
                    "p (c f) -> p c f", c=2),
                in_=coef_d[k, 1].rearrange("c p f -> p c f"))

        # ---- products: all DVE ----
        # per (half h, tap k): rr = s_re*c_re, ir = s_im*c_re,
        #                      nr = s_re*(-c_im), ni = s_im*(-c_im)
        # re += rr + ni ; im += ir - nr  (via +/-I PSUM accumulation)
        prods = [[dict() for _ in range(FS)] for _ in range(2)]
        pv = lambda t: t[:].rearrange("p (i f) -> p i f", f=NF)

        def make_prods(h, k):
            prods[h][k] = {key: ppool.tile([P, KC], f16, tag="prod",
                                           name=f"prod{h}{k}{key}")
                           for key in ("rr", "ir", "nr", "ni")}

        def product(h, k, key, r0, r1):
            s_plane = 0 if key in ("rr", "nr") else 1
            c_plane = 0 if key in ("rr", "ir") else 1
            scols = spec_cols(h, s_plane)[:, (k + r0) * NF:(k + r1) * NF]
            pcols = prods[h][k][key][:][:, r0 * NF:r1 * NF]
            if (k, h) == (0, 0):
                # ct00 is rowchunk-major in 4-row quarters: [j(4) c(2) i(4) f]
                cv = ct00[:].rearrange("p (j c i f) -> p j c i f",
                                       j=4, c=2, f=NF)[:, :, c_plane]
                if r1 - r0 == 4:
                    s = scols.rearrange("p (i f) -> p i f", f=NF)
                    dst = pcols.rearrange("p (i f) -> p i f", f=NF)
                    c = cv[:, r0 // 4]
                else:
                    jn = (r1 - r0) // 4
                    s = scols.rearrange("p (j i f) -> p j i f", j=jn, f=NF)
                    dst = pcols.rearrange("p (j i f) -> p j i f", j=jn, f=NF)
                    c = cv[:, r0 // 4:r1 // 4]
            else:
                ap = ctiles[(k, h)][:, c_plane * KC + r0 * NF:
                                    c_plane * KC + r1 * NF]
                s = scols.rearrange("p (i f) -> p i f", f=NF)
                dst = pcols.rearrange("p (i f) -> p i f", f=NF)
                c = ap.rearrange("p (i f) -> p i f", f=NF)
            nc.vector.tensor_mul(out=dst, in0=s, in1=c)

        # merged products for h1 taps 1+2: one op per key; the spec operand
        # is a hand-built AP [p, k(2), i(16), f(96)] whose tap dim strides
        # by ONE row (96) -- overlapping reads are legal for sources
        mprods = {}

        def mul_merged(key):
            s_plane = 0 if key in ("rr", "nr") else 1
            c_plane = 0 if key in ("rr", "ir") else 1
            base = spec_cols(1, s_plane).rearrange("p (r f) -> p r f", f=NF)
            sl = base[:, 1:1 + HALF, :]                          # tap-1 window
            p0 = sl.ap[0]
            s2 = _bass_rust.AP(
                sl.tensor, sl.offset,
                [[p0[0], p0[1]], [NF, 2], [NF, HALF], [1, NF]])
            cmv = ctm[:].rearrange("p (k c f) -> p k c f", k=2, c=2)[
                :, :, c_plane].rearrange("p k (i f) -> p k i f", f=NF)
            dst = mprods[key][:].rearrange("p (k i f) -> p k i f", k=2, f=NF)
            nc.vector.tensor_mul(out=dst, in0=s2, in1=cmv)

        mprods34 = {}

        def mul_merged34(key, r0, r1):
            s_plane = 0 if key in ("rr", "nr") else 1
            c_plane = 0 if key in ("rr", "ir") else 1
            base = spec_cols(1, s_plane).rearrange("p (r f) -> p r f", f=NF)
            sl = base[:, 3 + r0:3 + r1, :]                       # tap-3 window
            p0 = sl.ap[0]
            s2 = _bass_rust.AP(
                sl.tensor, sl.offset,
                [[p0[0], p0[1]], [NF, 2], [NF, r1 - r0], [1, NF]])
            cmv = ctm34[:].rearrange("p (k c f) -> p k c f", k=2, c=2)[
                :, :, c_plane].rearrange(
                "p k (i f) -> p k i f", f=NF)[:, :, r0:r1, :]
            dst = mprods34[key][:].rearrange(
                "p (k i f) -> p k i f", k=2, f=NF)[:, :, r0:r1, :]
            nc.vector.tensor_mul(out=dst, in0=s2, in1=cmv)

        # ---- PE accumulation + drains ----
        pss = {}

        def make_psum(h):
            for ch in (2 * h, 2 * h + 1):
                pss[ch] = (
                    pspool.tile([P, CW], f32, tag="psre", name=f"psre{ch}"),
                    pspool.tile([P, CW], f32, tag="psim", name=f"psim{ch}"),
                )

        def mm_src(h, k, key):
            if (k, h) in MERGED:
                ki = MERGED.index((k, h))
                return mprods[key][:][:, ki * KC:(ki + 1) * KC]
            if (k, h) in MERGED34:
                ki = MERGED34.index((k, h))
                return mprods34[key][:][:, ki * KC:(ki + 1) * KC]
            return prods[h][k][key][:]

        def mm(h, k, key, which, w, first, last, chunks=None):
            src = mm_src(h, k, key)
            for ch in chunks if chunks is not None else (2 * h, 2 * h + 1):
                off = (ch % 2) * CW
                ps = pss[ch][which]
                for a in range(0, CW, 512):
                    b = min(a + 512, CW)
                    nc.tensor.matmul(ps[:, a:b], w,
                                     src[:, off + a:off + b],
                                     start=first, stop=last)

        def mm_tap(h, k, first, last, chunks=None):
            mm(h, k, "rr", 0, ident, first, False, chunks)       # rr   -> re
            mm(h, k, "ir", 1, ident, first, False, chunks)       # ir   -> im
            mm(h, k, "ni", 0, ident, False, last, chunks)        # -ii  -> re
            mm(h, k, "nr", 1, identn, False, last, chunks)       # -ri  -> im

        def tail_chunk34(ch):
            # merged taps 3+4 products per chunk; psre stops at tap-4 ni,
            # psim stops at tap-4 nr
            r0, r1 = (0, CHUNK) if ch % 2 == 0 else (CHUNK, HALF)
            for key in ("rr", "ir", "ni", "nr"):
                mul_merged34(key, r0, r1)
            for key, which, w in (("rr", 0, ident), ("ir", 1, ident),
                                  ("ni", 0, ident), ("nr", 1, identn)):
                for k in (3, 4):
                    mm(1, k, key, which, w, False,
                       k == 4 and key in ("ni", "nr"), (ch,))

        def drain_store(ch, im_eng):
            ps_re, ps_im = pss[ch]
            ot = opool.tile([P, CHUNK * 2 * NF], f16, tag="out",
                            name=f"out{ch}")
            otv = ot[:].rearrange("p (i u) -> p i u", u=2 * NF)
            psv = lambda t: t[:].rearrange("p (i f) -> p i f", f=NF)
            nc.scalar.copy(out=otv[:, :, 0:NF], in_=psv(ps_re))
            if im_eng == "vector":
                nc.vector.tensor_copy(out=otv[:, :, NF:2 * NF],
                                      in_=psv(ps_im))
            else:
                nc.scalar.copy(out=otv[:, :, NF:2 * NF], in_=psv(ps_im))
            store_eng = nc.sync if ch >= 2 else nc.scalar
            store_eng.dma_start(
                out=outv[:, ch * CHUNK:(ch + 1) * CHUNK, :],
                in_=ot,
            )

        # ---- half 0: tap 0 in 4/4/8-row chunks (ramp), taps 1-4 full ----
        make_psum(0)
        make_prods(0, 0)
        for (r0, r1) in ((0, 4), (4, CHUNK)):
            for key in ("rr", "nr"):                             # loads 1-2 only
                product(0, 0, key, r0, r1)
        for (r0, r1) in ((0, 4), (4, CHUNK)):
            for key in ("ir", "ni"):                             # + s_im plane
                product(0, 0, key, r0, r1)
        for key in ("rr", "ir", "nr", "ni"):                     # + c0 rows 8:16
            product(0, 0, key, CHUNK, HALF)
        mm_tap(0, 0, True, False)
        for k in range(1, FS):
            make_prods(0, k)
            for key in ("rr", "ir", "nr", "ni"):
                product(0, k, key, 0, HALF)
            mm_tap(0, k, False, k == FS - 1)
        drain_store(0, "scalar")
        drain_store(1, "scalar")

        # ---- half 1 ----
        make_psum(1)
        make_prods(1, 0)
        for key in ("rr", "ir", "nr", "ni"):
            product(1, 0, key, 0, HALF)
        mm_tap(1, 0, True, False)
        # taps 1+2 merged
        for key in ("rr", "ir", "nr", "ni"):
            mprods[key] = pwpool.tile([P, 2 * KC], f16, tag="prodw",
                                      name=f"prodw{key}")
            mul_merged(key)
        for k in (1, 2):
            mm_tap(1, k, False, False)
        for key in ("rr", "ir", "nr", "ni"):
            mprods34[key] = pwpool.tile([P, 2 * KC], f16, tag="prodw",
                                        name=f"prodw34{key}")
        tail_chunk34(3)
        tail_chunk34(2)
        drain_store(3, "scalar")                                 # overlaps ch2 products
        # final chunk: drain + store in 4-row halves; ACT takes half 0,
        # DVE half 1, so both stores issue ~at the last product
        ps_re, ps_im = pss[2]
        ot = opool.tile([P, CHUNK * 2 * NF], f16, tag="out", name="out2")
        otv = ot[:].rearrange("p (i u) -> p i u", u=2 * NF)
        psv2 = lambda t: t[:].rearrange("p (i f) -> p i f", f=NF)
        for hh, re_eng, im_eng in ((0, "scalar", "scalar"),
                                   (1, "vector", "vector")):
            r0, r1 = hh * 4, (hh + 1) * 4
            for part, eng in (("re", re_eng), ("im", im_eng)):
                src = psv2(ps_re if part == "re" else ps_im)[:, r0:r1, :]
                dsl = otv[:, r0:r1, 0:NF] if part == "re" \
                    else otv[:, r0:r1, NF:2 * NF]
                if eng == "vector":
                    nc.vector.tensor_copy(out=dsl, in_=src)
                else:
                    nc.scalar.copy(out=dsl, in_=src)
            nc.sync.dma_start(
                out=outv[:, 2 * CHUNK + r0:2 * CHUNK + r1, :],
                in_=ot[:, r0 * 2 * NF:r1 * 2 * NF],
            )


def _build_nc():
    nc = bacc.Bacc("TRN2", target_bir_lowering=False, debug=False, num_devices=B)
    f16 = mybir.dt.float16
    spec_d = nc.dram_tensor("spec", [2, 2, P, SC], f16,
                            kind="ExternalInput").ap()
    coef_d = nc.dram_tensor("coefs", [FS, 2, 2, P, KC], f16,
                            kind="ExternalInput").ap()
    coef00_d = nc.dram_tensor("coef00", [4, P, JC], f16,
                              kind="ExternalInput").ap()
    const_d = nc.dram_tensor("consts", [P, 2 * P], f16,
                             kind="ExternalInput").ap()
    out_d = nc.dram_tensor("out", [T, 2 * NF], f16, kind="ExternalOutput").ap()
    with TileContext(nc) as tc:
        _body(nc, tc, spec_d, coef_d, coef00_d, const_d, out_d)
    nc.compile()
    return nc


def _in_maps(spec, coefs):
    spec = np.asarray(spec, dtype=np.float32)
    coefs = np.asarray(coefs, dtype=np.float32)
    consts = np.concatenate(
        [np.eye(P, dtype=np.float16), -np.eye(P, dtype=np.float16)], axis=1
    )
    maps = []
    for b in range(B):
        # spec window rows 32p-4..32p+32 per partition, then pieces
        # [0:20) and [16:36): [2(piece), 2(plane), 128, 20*96]
        s_lo = spec[b, 0, :, :NF, :].astype(np.float16)          # [4096, 96, 2]
        blk = s_lo.reshape(P, TB, NF, 2)
        win = np.zeros((P, WIN, NF, 2), dtype=np.float16)
        win[:, HL:] = blk
        win[1:, :HL] = blk[:-1, TB - HL:]
        wpl = win.transpose(3, 0, 1, 2)                          # [2,P,36,96]
        spec_pk = np.stack(
            [wpl[:, :, 0:SROWS], wpl[:, :, HALF:HALF + SROWS]], axis=0
        ).reshape(2, 2, P, SC)
        spec_pk = np.ascontiguousarray(spec_pk)

        # coefs plane-major: [5(tap), 2(half), 2(plane), 128, 16*96]
        c = coefs[b].reshape(FS, P, 2, HALF, NF, 2)
        cpk = c.transpose(0, 2, 5, 1, 3, 4).copy()  # [5,2(h),2(c),P,16,96]
        cpk[:, :, 1] *= -1.0
        coef_pk = cpk.astype(np.float16).reshape(FS, 2, 2, P, KC)

        # tap0-h0 duplicated rowchunk-major in 4-row quarters:
        # [4(j), P, 2(c)*4*96]
        c00 = cpk[0, 0].reshape(2, P, 4, 4, NF)                  # [c,P,j,4,96]
        coef00_pk = np.ascontiguousarray(
            c00.transpose(2, 1, 0, 3, 4)).reshape(4, P, JC).astype(np.float16)
        maps.append({"spec": spec_pk, "coefs": coef_pk,
                     "coef00": coef00_pk, "consts": consts})
    return maps


def kernel(spec, coefs):
    global _nc_cache
    if _nc_cache is None:
        _nc_cache = _build_nc()
    res = run_bass_kernel_spmd(_nc_cache, _in_maps(spec, coefs),
                               core_ids=list(range(B)))
    out = np.asarray(spec, dtype=np.float32).copy()              # hi band
    for b in range(B):
        lo = res.results[b]["out"].astype(np.float32)            # [4096, 192]
        out[b, 0, :, :NF, 0] = lo[:, :NF]
        out[b, 0, :, :NF, 1] = lo[:, NF:]
    return out
